# revision 7
# baseline (speedup 1.0000x reference)
"""EGNN vector-field kernel for Trainium2, SPMD over 8 NeuronCores (v2).

Sharding: 2 batches x 4 cores; each core owns 1024 query targets (8192 edges).
Per layer, each core computes its shard of P1 = h @ eW1[:C] (1024 targets +
128 grid nodes) plus a bf16 hi/lo split of node coordinates into a 640-column
table row [P1(512) | xhi(3) xlo(3) | pad], AllGathers the table (groups of 4),
then runs the edge pipeline per 2048-edge chunk with a single transposed
dma_gather per chunk serving both the P1[row] features ([H, e] layout) and
source coordinates. P2[col] + eb1 is added via a broadcast (step-0) access
pattern from an SBUF-resident transposed P2. dist/dirn run fp32 in
edge-on-partition layout with a DVE Newton rsqrt (no ACT table switches).
Segment means use the contiguous-8 edge structure. Node MLP on PE; outputs
assembled per-core and stitched on host.
"""

import os
import numpy as np

import concourse.bacc as bacc
import concourse.bass as bass
import concourse.mybir as mybir
import concourse.tile as tile
from concourse.bass_utils import run_bass_kernel_spmd

F32 = mybir.dt.float32
BF16 = mybir.dt.bfloat16
I16 = mybir.dt.int16
I32 = mybir.dt.int32
AF = mybir.ActivationFunctionType
ALU = mybir.AluOpType

B = 2
NP = 4096
NG = 512
T = 4608
KNN = 8
C = 512
HD = 512
A = 5
E_B = T * KNN

NCORES = 8
GPC = 4
TGT = NP // GPC        # 1024
EC = TGT * KNN         # 8192
ECH = 2048
NCHUNK = EC // ECH
GRID_SH = NG // GPC    # 128
SHARD = TGT + GRID_SH  # 1152
RS = SHARD             # table rank stride (rows)
ROWW = 640             # table row width (bf16): 512 P1 + 3 xhi + 3 xlo + pad
NLAYERS = 4

RSQRT_MAGIC = 0x5F3759DF

LAST_RESULTS = None


def _perm_row(j):
    if j < NP:
        r, off = j // TGT, j % TGT
        return r * RS + off
    g = j - NP
    r = g // GRID_SH
    return r * RS + TGT + (g % GRID_SH)


_PERM = np.array([_perm_row(j) for j in range(T)], dtype=np.int16)


def _wrap_idx(idx_flat):
    idx_flat = idx_flat.astype(np.int16)
    w = idx_flat.reshape(-1, 16).T
    return np.tile(w, (8, 1))


def _bf(x):
    import ml_dtypes
    return np.asarray(x, dtype=np.float32).astype(ml_dtypes.bfloat16)


def _prep_host(query_points, codes, grid_points, edge_index, params):
    import ml_dtypes
    qp = np.asarray(query_points, dtype=np.float32)
    cod = np.asarray(codes, dtype=np.float32)
    gp = np.asarray(grid_points, dtype=np.float32)
    ei = np.asarray(edge_index, dtype=np.int64)
    row, col = ei[0], ei[1]

    exp_col = np.concatenate(
        [np.repeat(np.arange(T), KNN) + b * T for b in range(B)])
    if not np.array_equal(col, exp_col):
        raise ValueError("edge_index col does not have the expected "
                         "repeat(arange(T), K) block structure")
    for b in range(B):
        rb = row[b * E_B:(b + 1) * E_B]
        if rb.min() < b * T or rb.max() >= (b + 1) * T:
            raise ValueError("edge rows cross batch boundary")

    layers = list(params["layers"]) + [params["field"]]
    assert len(layers) == NLAYERS

    we1a = np.stack([_bf(np.asarray(p["eW1"])[:C, :]) for p in layers])
    we1b = np.stack([_bf(np.asarray(p["eW1"])[C:2 * C, :]) for p in layers])
    we2 = np.stack([_bf(np.asarray(p["eW2"])) for p in layers])
    wn1 = []
    wn2 = []
    for p in layers:
        n1 = np.asarray(p["nW1"], dtype=np.float32).copy()
        n1[C:, :] = n1[C:, :] / 8.0
        wn1.append(_bf(n1))
        wn2.append(_bf(np.asarray(p["nW2"])))
    wn1 = np.stack(wn1)
    wn2 = np.stack(wn2)
    wc = np.zeros((NLAYERS, HD, 8), np.float32)
    for li, p in enumerate(layers):
        cw = np.asarray(p["cW"], dtype=np.float32)
        wc[li, :, :cw.shape[1]] = cw
    wc = _bf(wc)

    def col16(vecs):
        out = np.zeros((128, 4 * len(vecs)), np.float32)
        for li, v in enumerate(vecs):
            out[:, li * 4:(li + 1) * 4] = \
                np.asarray(v, np.float32).reshape(4, 128).T
        return out

    wd = col16([np.asarray(p["eW1"])[2 * C, :] for p in layers])
    be1 = col16([p["eb1"] for p in layers])
    be2 = col16([p["eb2"] for p in layers])
    bn1 = col16([p["nb1"] for p in layers])
    bn2 = col16([p["nb2"] for p in layers])
    bc = np.zeros((128, NLAYERS), np.float32)
    for li, p in enumerate(layers):
        cb = np.asarray(p["cb"], np.float32)
        bc[:cb.shape[0], li] = cb

    sx = np.zeros((128, 16), np.float32)
    for p in range(128):
        sx[p, p // 8] = 0.125
    ones_bf = _bf(np.ones((1, 128), np.float32))

    shared = dict(we1a=we1a, we1b=we1b, we2=we2, wn1=wn1, wn2=wn2, wc=wc,
                  wd=wd, be1=be1, be2=be2, bn1=bn1, bn2=bn2, bc=bc,
                  sx=sx, ones_bf=ones_bf,
                  identb=_bf(np.eye(128, dtype=np.float32)))

    def hilo(x):
        hi = x.astype(ml_dtypes.bfloat16)
        lo = (x - hi.astype(np.float32)).astype(ml_dtypes.bfloat16)
        return hi, lo

    in_maps = []
    for core in range(NCORES):
        b, c = divmod(core, GPC)
        codesT = np.ascontiguousarray(
            cod[b].T[:, c * GRID_SH:(c + 1) * GRID_SH])

        tsel = np.arange(TGT) + c * TGT
        tij = tsel.reshape(64, 16)
        xq0 = np.ascontiguousarray(qp[b][tij.T])      # [16, 64, 3]

        ghi, glo = hilo(gp[c * GRID_SH:(c + 1) * GRID_SH])  # [128, 3]
        gxhl = np.ascontiguousarray(np.concatenate([ghi, glo], axis=1))

        g0 = b * E_B + c * EC
        rowl = (row[g0:g0 + EC] - b * T).astype(np.int64)
        idx_row = _wrap_idx(_PERM[rowl])

        m = dict(shared)
        m.update(codesT=codesT, xq0=np.ascontiguousarray(xq0.reshape(16, 192)),
                 gxhl=gxhl, idx_row=idx_row)
        in_maps.append(m)
    return in_maps


def _build_program():
    nc = bacc.Bacc("TRN2", target_bir_lowering=False, debug=False,
                   num_devices=NCORES)

    def din(name, shape, dtype):
        return nc.dram_tensor(name, list(shape), dtype, kind="ExternalInput")

    codesT = din("codesT", (C, GRID_SH), F32)
    xq0 = din("xq0", (16, 192), F32)
    gxhl = din("gxhl", (GRID_SH, 6), BF16)
    idx_row = din("idx_row", (128, EC // 16), I16)
    we1a = din("we1a", (NLAYERS, C, HD), BF16)
    we1b = din("we1b", (NLAYERS, C, HD), BF16)
    we2 = din("we2", (NLAYERS, HD, HD), BF16)
    wn1 = din("wn1", (NLAYERS, C + HD, HD), BF16)
    wn2 = din("wn2", (NLAYERS, HD, C), BF16)
    wc = din("wc", (NLAYERS, HD, 8), BF16)
    wd = din("wd", (128, 16), F32)
    be1 = din("be1", (128, 16), F32)
    be2 = din("be2", (128, 16), F32)
    bn1 = din("bn1", (128, 16), F32)
    bn2 = din("bn2", (128, 16), F32)
    bc = din("bc", (128, NLAYERS), F32)
    sx = din("sx", (128, 16), F32)
    ones_bf = din("ones_bf", (1, 128), BF16)
    identb = din("identb", (128, 128), BF16)

    out_d = nc.dram_tensor("out", [16, 64, 15], F32, kind="ExternalOutput")

    RG = [[0, 1, 2, 3], [4, 5, 6, 7]]

    with tile.TileContext(nc) as tc:
        import contextlib
        with contextlib.ExitStack() as ctx:
            dram = ctx.enter_context(tc.tile_pool(name="dram", bufs=1, space="DRAM"))
            sb1 = ctx.enter_context(tc.tile_pool(name="sb1", bufs=1))
            sbw = ctx.enter_context(tc.tile_pool(name="sbw", bufs=1))
            pA = ctx.enter_context(tc.tile_pool(name="pA", bufs=2))
            pZ = ctx.enter_context(tc.tile_pool(name="pZ", bufs=2))
            pM = ctx.enter_context(tc.tile_pool(name="pM", bufs=2))
            pS = ctx.enter_context(tc.tile_pool(name="pS", bufs=2))
            pS1 = ctx.enter_context(tc.tile_pool(name="pS1", bufs=1))
            stg = ctx.enter_context(tc.tile_pool(name="stg", bufs=2))
            pbig = ctx.enter_context(tc.tile_pool(name="pbig", bufs=1, space="PSUM"))
            psm = ctx.enter_context(tc.tile_pool(name="psm", bufs=2, space="PSUM"))
            pst = ctx.enter_context(tc.tile_pool(name="pst", bufs=1, space="PSUM"))
            psb = ctx.enter_context(tc.tile_pool(name="psb", bufs=1, space="PSUM"))

            agin = dram.tile([RS, ROWW], BF16, tag="agin")
            agout = dram.tile([GPC * RS, ROWW], BF16, tag="agout")
            xq8 = dram.tile([EC, 3], F32, tag="xq8")
            coefd = dram.tile([8, ECH], BF16, tag="coefd")
            xbnc = dram.tile([6, ECH], BF16, tag="xbnc")

            hT = sb1.tile([128, 4, SHARD], F32, tag="hT")
            hbf = sb1.tile([128, 4, SHARD], BF16, tag="hbf")
            p2t = sb1.tile([128, 4, TGT], BF16, tag="p2t")
            xq = sb1.tile([16, 64, 3], F32, tag="xq")
            qp_sb = sb1.tile([16, 64, 3], F32, tag="qp")
            dx_sb = sb1.tile([16, 64, 3], F32, tag="dx")
            dx5_sb = sb1.tile([16, 64, 15], F32, tag="dx5")
            m_aggr = sb1.tile([128, 4, TGT], BF16, tag="maggr")
            wd_sb = sb1.tile([128, 16], F32, tag="wd")
            be1_sb = sb1.tile([128, 16], F32, tag="be1")
            be2_sb = sb1.tile([128, 16], F32, tag="be2")
            bn1_sb = sb1.tile([128, 16], F32, tag="bn1")
            bn2_sb = sb1.tile([128, 16], F32, tag="bn2")
            bc_sb = sb1.tile([128, NLAYERS], F32, tag="bc")
            sx_sb = sb1.tile([128, 16], F32, tag="sx")
            ones_sb = sb1.tile([1, 128], BF16, tag="ones")
            ident_bf = sb1.tile([128, 128], BF16, tag="identb")
            ixr_sb = sb1.tile([128, EC // 16], I16, tag="ixr")
            xhl_sb = sb1.tile([16, 64, 6], BF16, tag="xhl")

            wa_sb = sbw.tile([128, 4, HD], BF16, tag="wa")
            wb_sb = sbw.tile([128, 4, HD], BF16, tag="wb")
            we2_sb = sbw.tile([128, 4, HD], BF16, tag="we2")
            wc_sb = sbw.tile([128, 4, 8], BF16, tag="wc")
            wn1_sb = sbw.tile([128, 8, HD], BF16, tag="wn1")
            wn2_sb = sbw.tile([128, 4, C], BF16, tag="wn2")

            sync = nc.sync
            gps = nc.gpsimd
            vec = nc.vector
            act = nc.scalar
            pe = nc.tensor

            for t_, s_ in [(wd_sb, wd), (be1_sb, be1), (be2_sb, be2),
                           (bn1_sb, bn1), (bn2_sb, bn2), (bc_sb, bc),
                           (sx_sb, sx), (ones_sb, ones_bf),
                           (ident_bf, identb), (ixr_sb, idx_row)]:
                sync.dma_start(t_[:], s_[:])
            sync.dma_start(qp_sb[:], xq0[:].rearrange("g (i c) -> g i c", c=3))
            sync.dma_start(xq[:], xq0[:].rearrange("g (i c) -> g i c", c=3))
            vec.memset(hT[:, :, 0:TGT], 0.0)
            for k in range(4):
                sync.dma_start(hT[:, k, TGT:SHARD],
                               codesT[k * 128:(k + 1) * 128, :])
            vec.tensor_copy(hbf[:], hT[:])
            # constant grid x hi/lo into agin rows [TGT, SHARD), cols 512:518
            sync.dma_start(agin[TGT:SHARD, 512:518], gxhl[:])

            def load_wab(li):
                for k in range(4):
                    sync.dma_start(wa_sb[:, k, :], we1a[li, k * 128:(k + 1) * 128, :])
                    sync.dma_start(wb_sb[:, k, :], we1b[li, k * 128:(k + 1) * 128, :])

            def load_wrest(li):
                for k in range(4):
                    sync.dma_start(we2_sb[:, k, :], we2[li, k * 128:(k + 1) * 128, :])
                    sync.dma_start(wc_sb[:, k, :], wc[li, k * 128:(k + 1) * 128, :])
                    sync.dma_start(wn2_sb[:, k, :], wn2[li, k * 128:(k + 1) * 128, :])
                if li < 3:
                    for k in range(8):
                        sync.dma_start(wn1_sb[:, k, :], wn1[li, k * 128:(k + 1) * 128, :])

            def write_own_x():
                """xq (fp32) -> hi/lo bf16 -> agin[0:TGT, 512:518]."""
                vec.tensor_copy(xhl_sb[:, :, 0:3], xq[:])
                lo32 = pS1.tile([16, 64, 3], F32, tag="lo32")
                vec.tensor_sub(lo32[:], xq[:], xhl_sb[:, :, 0:3])
                vec.tensor_copy(xhl_sb[:, :, 3:6], lo32[:])
                dst = agin[0:TGT, 512:518].rearrange("(i g) c -> g i c", g=16)
                sync.dma_start(dst, xhl_sb[:])

            def p1_phase(li):
                for mc in range(SHARD // 128):
                    pp = psm.tile([128, 512], F32, tag="psA")
                    for k in range(4):
                        pe.matmul(pp[:], hbf[:, k, mc * 128:(mc + 1) * 128],
                                  wa_sb[:, k, :], start=(k == 0), stop=(k == 3))
                    st = stg.tile([128, 512], BF16, tag="p1st")
                    vec.tensor_copy(st[:], pp[:])
                    sync.dma_start(agin[mc * 128:(mc + 1) * 128, 0:512], st[:])

            def ag_phase():
                gps.collective_compute(
                    "AllGather", ALU.bypass, replica_groups=RG,
                    ins=[agin[:].opt()], outs=[agout[:].opt()])

            def xq8_rebuild():
                for k in range(KNN):
                    dst = xq8[:].rearrange("(t k) c -> k t c", k=KNN)[k] \
                        .rearrange("(i g) c -> g i c", g=16)
                    sync.dma_start(dst, xq[:])

            def p2_phase(li):
                """P2T [H, tgt] bf16 + eb1, SBUF-resident."""
                for mj in range(4):
                    for s in range(TGT // 512):
                        pp = psm.tile([128, 512], F32, tag="psA")
                        for k in range(4):
                            pe.matmul(pp[:], wb_sb[:, k, mj * 128:(mj + 1) * 128],
                                      hbf[:, k, s * 512:(s + 1) * 512],
                                      start=(k == 0), stop=(k == 3))
                        vec.tensor_scalar_add(
                            p2t[:, mj, s * 512:(s + 1) * 512], pp[:],
                            be1_sb[:, li * 4 + mj:li * 4 + mj + 1])

            def edge_chunk(li, ch, outx):
                isl = slice(ch * (ECH // 16), (ch + 1) * (ECH // 16))
                t0 = ch * (ECH // KNN)
                p1g = pA.tile([128, 5, ECH], BF16, tag="p1g")
                gps.dma_gather(p1g[:], agout[:], ixr_sb[:, isl], ECH, ECH,
                               ROWW, transpose=True, single_packet=False)

                # --- x pipeline (edge-on-partition, fp32) ---
                sync.dma_start(xbnc[:], p1g[0:6, 4, :])
                xrs = pS.tile([128, 16, 6], BF16, tag="xrs")
                for c6 in range(6):
                    sync.dma_start(
                        xrs[:, :, c6],
                        xbnc[c6, :].rearrange("(i p) -> p i", p=128))
                xcv = pS.tile([128, 16, 3], F32, tag="xcv")
                sync.dma_start(
                    xcv[:],
                    xq8[ch * ECH:(ch + 1) * ECH, :]
                    .rearrange("(i p) c -> p i c", p=128))
                relv = pS.tile([128, 16, 3], F32, tag="relv")
                vec.tensor_add(relv[:], xrs[:, :, 0:3], xrs[:, :, 3:6])
                vec.tensor_sub(relv[:], relv[:], xcv[:])
                rel2 = pS.tile([128, 16, 3], F32, tag="rel2")
                vec.tensor_mul(rel2[:], relv[:], relv[:])
                d2 = pS.tile([128, 16], F32, tag="d2")
                vec.reduce_sum(d2[:], rel2[:], axis=mybir.AxisListType.X)
                # Newton rsqrt: rcp ~= 1/sqrt(d2)
                rcp = pS.tile([128, 16], F32, tag="rcp")
                tmpn = pS.tile([128, 16], F32, tag="tmpn")
                vec.tensor_scalar(rcp[:].bitcast(I32), d2[:].bitcast(I32),
                                  1, None, ALU.logical_shift_right)
                vec.tensor_scalar(rcp[:].bitcast(I32), rcp[:].bitcast(I32),
                                  -1, RSQRT_MAGIC, ALU.mult, ALU.add)
                for _ in range(2):
                    vec.tensor_mul(tmpn[:], rcp[:], rcp[:])
                    vec.tensor_mul(tmpn[:], tmpn[:], d2[:])
                    vec.tensor_scalar(tmpn[:], tmpn[:], -0.5, 1.5,
                                      ALU.mult, ALU.add)
                    vec.tensor_mul(rcp[:], rcp[:], tmpn[:])
                dist = pS.tile([128, 16], F32, tag="dist")
                vec.tensor_mul(dist[:], d2[:], rcp[:])
                dirn = pS.tile([128, 16, 3], F32, tag="dirn")
                for c3 in range(3):
                    vec.tensor_mul(dirn[:, :, c3], relv[:, :, c3], rcp[:])

                # dist -> [1, ECH] bf16 row via PE transpose + flatten DMA
                distb16 = pS.tile([128, 16], BF16, tag="distb16")
                vec.tensor_copy(distb16[:], dist[:])
                dtp = pst.tile([16, 128], BF16, tag="psT")
                pe.transpose(dtp[:], distb16[:], ident_bf[:])
                dts = pS1.tile([16, 128], BF16, tag="dts")
                vec.tensor_copy(dts[:], dtp[:])
                drow = pS1.tile([1, ECH], BF16, tag="drow")
                sync.dma_start(
                    drow[0:1, :].rearrange("a (i p) -> a i p", p=128), dts[:])
                distb = pS1.tile([128, ECH], BF16, tag="distb")
                for s in range(ECH // 512):
                    dbp = psm.tile([128, 512], F32, tag="psA")
                    pe.matmul(dbp[:], ones_sb[:],
                              drow[:, s * 512:(s + 1) * 512])
                    vec.tensor_copy(distb[:, s * 512:(s + 1) * 512], dbp[:])

                # z = silu(P1g + P2T[col]-bcast + wd*dist)  (eb1 inside P2T)
                z = pZ.tile([128, 4, ECH], BF16, tag="z")
                vec.tensor_add(
                    z[:].rearrange("p j (t o) -> p j t o", o=KNN),
                    p1g[:, 0:4, :].rearrange("p j (t o) -> p j t o", o=KNN),
                    p2t[:, :, t0:t0 + ECH // KNN]
                    .broadcast_to([128, 4, ECH // KNN, KNN]))
                for j in range(4):
                    vec.scalar_tensor_tensor(
                        z[:, j, :], distb[:], wd_sb[:, li * 4 + j:li * 4 + j + 1],
                        z[:, j, :], ALU.mult, ALU.add)
                act.activation(z[:].rearrange("p j e -> p (j e)"),
                               z[:].rearrange("p j e -> p (j e)"), AF.Silu)

                # m = silu(z @ eW2 + eb2)
                m = pM.tile([128, 4, ECH], BF16, tag="m")
                for j in range(4):
                    mp = pbig.tile([128, ECH], F32, tag="mp")
                    for s in range(ECH // 512):
                        for k in range(4):
                            pe.matmul(mp[:, s * 512:(s + 1) * 512],
                                      we2_sb[:, k, j * 128:(j + 1) * 128],
                                      z[:, k, s * 512:(s + 1) * 512],
                                      start=(k == 0), stop=(k == 3))
                    act.activation(m[:, j, :], mp[:], AF.Silu,
                                   bias=be2_sb[:, li * 4 + j:li * 4 + j + 1])

                if li < 3:
                    for j in range(4):
                        a1 = pS1.tile([128, 256, 4], BF16, tag="a1")
                        mv = m[:, j, :].rearrange("p (t k) -> p t k", k=8)
                        vec.tensor_add(a1[:], mv[:, :, 0:4], mv[:, :, 4:8])
                        a2 = pS1.tile([128, 256, 2], BF16, tag="a2")
                        vec.tensor_add(a2[:], a1[:, :, 0:2], a1[:, :, 2:4])
                        vec.tensor_add(
                            m_aggr[:, j, ch * 256:(ch + 1) * 256],
                            a2[:, :, 0], a2[:, :, 1])

                for s in range(ECH // 512):
                    cp = psm.tile([128, 512], F32, tag="psA")
                    for k in range(4):
                        pe.matmul(cp[0:8, :], wc_sb[:, k, :],
                                  m[:, k, s * 512:(s + 1) * 512],
                                  start=(k == 0), stop=(k == 3))
                    coefsb = pS1.tile([8, 512], BF16, tag="coefsb")
                    vec.tensor_scalar_add(coefsb[:],
                                          cp[0:8, :], bc_sb[0:8, li:li + 1])
                    sync.dma_start(coefd[:, s * 512:(s + 1) * 512], coefsb[:])
                crs = pS1.tile([128, 16, 8], BF16, tag="crs")
                for o in range(outx):
                    sync.dma_start(
                        crs[:, :, o],
                        coefd[o, :].rearrange("(i p) -> p i", p=128))

                cdir = pS1.tile([128, 16, 3 * outx], F32, tag="cdir")
                for o in range(outx):
                    for c3 in range(3):
                        vec.tensor_mul(cdir[:, :, o * 3 + c3],
                                       crs[:, :, o], dirn[:, :, c3])
                dxp = psb.tile([16, 16 * 3 * outx], F32, tag="psB")
                pe.matmul(dxp[:], sx_sb[:],
                          cdir[:].rearrange("p i v -> p (i v)"))
                if outx == 1:
                    vec.tensor_copy(
                        dx_sb[:, ch * 16:(ch + 1) * 16, :],
                        dxp[:].rearrange("g (i v) -> g i v", v=3))
                else:
                    vec.tensor_copy(
                        dx5_sb[:, ch * 16:(ch + 1) * 16, :],
                        dxp[:].rearrange("g (i v) -> g i v", v=15))

            def node_mlp(li):
                n1 = pA.tile([128, 5, ECH], BF16, tag="p1g")  # reuse slot
                for mj in range(4):
                    for s in range(TGT // 512):
                        pp = psm.tile([128, 512], F32, tag="psA")
                        for k in range(4):
                            pe.matmul(pp[:], wn1_sb[:, k, mj * 128:(mj + 1) * 128],
                                      hbf[:, k, s * 512:(s + 1) * 512],
                                      start=(k == 0), stop=False)
                        for k in range(4):
                            pe.matmul(pp[:], wn1_sb[:, 4 + k, mj * 128:(mj + 1) * 128],
                                      m_aggr[:, k, s * 512:(s + 1) * 512],
                                      start=False, stop=(k == 3))
                        act.activation(n1[:, mj, s * 512:(s + 1) * 512], pp[:],
                                       AF.Silu,
                                       bias=bn1_sb[:, li * 4 + mj:li * 4 + mj + 1])
                for mj in range(4):
                    for s in range(TGT // 512):
                        pp = psm.tile([128, 512], F32, tag="psA")
                        for k in range(4):
                            pe.matmul(pp[:], wn2_sb[:, k, mj * 128:(mj + 1) * 128],
                                      n1[:, k, s * 512:(s + 1) * 512],
                                      start=(k == 0), stop=(k == 3))
                        vec.scalar_tensor_tensor(
                            hT[:, mj, s * 512:(s + 1) * 512], pp[:],
                            bn2_sb[:, li * 4 + mj:li * 4 + mj + 1],
                            hT[:, mj, s * 512:(s + 1) * 512],
                            ALU.add, ALU.add)

            # ================= program =================
            load_wab(0)
            write_own_x()
            p1_phase(0)
            ag_phase()
            xq8_rebuild()
            for li in range(NLAYERS):
                outx = 1 if li < 3 else A
                load_wrest(li)
                p2_phase(li)
                for ch in range(NCHUNK):
                    edge_chunk(li, ch, outx)
                if li < 3:
                    node_mlp(li)
                    vec.tensor_copy(hbf[:], hT[:])
                    load_wab(li + 1)
                    p1_phase(li + 1)
                    vec.tensor_add(xq[:], xq[:], dx_sb[:])
                    write_own_x()
                    ag_phase()
                    xq8_rebuild()
                else:
                    base = pS1.tile([16, 64, 3], F32, tag="base")
                    vec.tensor_sub(base[:], xq[:], qp_sb[:])
                    for o in range(A):
                        vec.tensor_add(dx5_sb[:, :, o * 3:(o + 1) * 3],
                                       base[:], dx5_sb[:, :, o * 3:(o + 1) * 3])
                    sync.dma_start(out_d[:], dx5_sb[:])

    nc.compile()
    return nc


_NC_CACHE = None


def _get_program():
    global _NC_CACHE
    if _NC_CACHE is None:
        _NC_CACHE = _build_program()
    return _NC_CACHE


def kernel(query_points, codes, grid_points, edge_index, params):
    global LAST_RESULTS
    in_maps = _prep_host(query_points, codes, grid_points, edge_index, params)
    nc = _get_program()

    trace = os.environ.get("KERNEL_TRACE", "0") == "1"
    if os.environ.get("KERNEL_SIM", "0") == "1":
        from concourse.bass_interp import MultiCoreSim
        sim = MultiCoreSim(nc, num_cores=NCORES, trace=False,
                           require_finite=False, require_nnan=False)
        for ci in range(NCORES):
            core = sim.cores[ci]
            for name, arr in in_maps[ci].items():
                core.tensor(name)[:] = np.asarray(arr)
        sim.simulate(check_with_hw=False)
        results = [{"out": np.array(sim.cores[ci].tensor("out"))}
                   for ci in range(NCORES)]
        LAST_RESULTS = None
    else:
        res = run_bass_kernel_spmd(
            nc, in_maps, core_ids=list(range(NCORES)), trace=trace)
        LAST_RESULTS = res
        results = res.results

    out = np.zeros((B, NP, A, 3), np.float32)
    for core in range(NCORES):
        b, c = divmod(core, GPC)
        o = results[core]["out"].reshape(16, 64, 5, 3)
        o = o.transpose(1, 0, 2, 3).reshape(TGT, A, 3)
        out[b, c * TGT:(c + 1) * TGT] = o
    return out


if __name__ == "__main__":
    print("kernel module ok")
